# revision 30
# baseline (speedup 1.0000x reference)
"""Trainium2 Bass kernel for nn_Attention_41729902248209.

8-head attention block: x (8, 512, 32, 32) -> QKV proj -> softmax attention
-> out proj + residual. Data-parallel over batch: one batch element per
NeuronCore (8 cores).

Per-core dataflow (n = 1024 tokens, cin = 512, H = 8 heads, D = 64):
  - feature-on-partitions everywhere (no on-chip transposes):
      qT, kT : (f' = 64h+d on partitions, n free) fp16 [head pairs share tiles]
      v8     : (n on partitions, 66h+e free, fp8e4) with a ones column per
               head at e=64 (integrates the softmax denominator in attnv)
      scoresT: (j partitions, i free) fp32 PSUM = k @ qT [2 heads concurrent
               via tile_position quadrants]
      pT     : exp(scoresT - 3) in fp8e4, straight off PSUM on ACT (the -3
               keeps exp under the TRN fp8e4 240-max; softmax-invariant)
      outT~  : attnv via fp8 DoubleRow (j-tile pairs, 2x PE rate): rows 0:64
               unscaled outT, row 64 denominator
      os8    : outT * (1/denom) in fp8e4 -> out proj rhs
      yT     : W_lastT.T @ os8 (fp8 DoubleRow over k-tile pairs) + (x + b')
  - biases: b_q/b_k fused into the qk evac (DVE tensor_scalar_add);
    b_v folded host-side into the residual (softmax weights sum to 1, so
    Sum p (v + bv) = Sum p v + bv -> b' = b_last + W_last @ b_v);
    softmax scale 1/8 folded into W_q host-side.
  - denominators: reciprocal_approx_fast straight off the PSUM denom row
    (p64) -> partition 0, then one GPSIMD partition_broadcast and one DVE
    multiply off PSUM into os8. No per-(h,c) full reciprocal / extra copies.
  - emission is software-pipelined: the ACT exp stream (64 x ~1us) paces the
    kernel; qk quarters / v tiles / attnv DoubleRow units / denominator
    chains drain behind it so PE+DVE stay fed without stalling ACT.
"""

import numpy as np
import ml_dtypes

import concourse.mybir as mybir
import concourse.tile as tile
from concourse import bacc
from concourse.bass_utils import run_bass_kernel_spmd

F8 = mybir.dt.float8e4
F16 = mybir.dt.float16
F32 = mybir.dt.float32

BS = 8
H = 8
D = 64
CIN = 512
N = 1024
NK = CIN // 128  # contraction tiles for cin
NJT = N // 128  # j tiles
NCH = N // 512  # i chunks of 512
VR = D + 2  # 66: per-head v block [v_h (64) | 1 | pad] (pad keeps stride%16==0)
VROW = H * VR  # 528 per j-tile
EXPC = 3.0  # exp shift: p = exp(s - EXPC); cancels in softmax

AF = mybir.ActivationFunctionType
ALU = mybir.AluOpType
PM = mybir.MatmulPerfMode

# HW-bisect switches. reciprocal_approx_fast produces garbage on real HW
# (custom-DVE ucode diverges from CoreSim) — keep the plain reciprocal.
USE_DR_ATTNV = True  # DoubleRow fp8 attnv (M=65) vs plain fp8 matmuls
USE_DR_OUTPROJ = True  # DoubleRow fp8 outproj vs plain fp8 matmuls
USE_RECIP_FAST = False  # reciprocal_approx_fast is HW-broken; use reciprocal


def _emit(tc, d, sb, ps):
    nc = tc.nc

    x16_sb = sb.tile([128, NK * N], F16, tag="x16")
    xr_sb = sb.tile([128, NK * N], F32, tag="xr")
    wq_sb = sb.tile([128, NK * 512], F16, tag="wq")
    wk_sb = sb.tile([128, NK * 512], F16, tag="wk")
    wv_sb = sb.tile([128, NK * 512], F16, tag="wv")
    wl8_sb = sb.tile([128, NK * 512], F8, tag="wl8")
    bqk_sb = sb.tile([128, 8], F32, tag="bqk")
    expb_sb = sb.tile([128, 1], F32, tag="expb")
    qT_sb = sb.tile([128, 4 * N], F16, tag="qT")
    kT_sb = sb.tile([128, 4 * N], F16, tag="kT")
    v8_sb = sb.tile([128, NJT * VROW], F8, tag="v8")
    os_sb = sb.tile([128, NK * N], F8, tag="os8")
    yA_sb = sb.tile([128, NK * N], F32, tag="yA")

    # --- input DMAs. The DMA engines drain one transfer at a time in
    # roughly issue order, so arrival ORDER is the whole game for fill:
    # bqk first (tiny, gates the first evac), then per-k {wq, x16, wk}
    # trios so the first qk quarters complete ASAP, then wv/wl8, then the
    # big xr (only needed by the final residual). Issues are spread across
    # three idle DGE queues. dram (k*128+p, n) -> sbuf cols [W*k, W*k+W).
    nc.sync.dma_start(bqk_sb[:], d["bqk"].ap())
    for k in range(NK):
        r = slice(128 * k, 128 * k + 128)
        nc.sync.dma_start(wq_sb[:, 512 * k : 512 * k + 512], d["wq"].ap()[r, :])
        nc.gpsimd.dma_start(x16_sb[:, N * k : N * k + N], d["x16"].ap()[r, :])
        nc.scalar.dma_start(wk_sb[:, 512 * k : 512 * k + 512], d["wk"].ap()[r, :])
    nc.sync.dma_start(
        wv_sb[:].rearrange("p (k n) -> p k n", k=NK),
        d["wv"].ap().rearrange("(k p) n -> p k n", k=NK),
    )
    nc.scalar.dma_start(
        wl8_sb[:].rearrange("p (k n) -> p k n", k=NK),
        d["wl8"].ap().rearrange("(k p) n -> p k n", k=NK),
    )
    for k in range(NK):
        r = slice(128 * k, 128 * k + 128)
        nc.gpsimd.dma_start(xr_sb[:, N * k : N * k + N], d["xr"].ap()[r, :])
    nc.vector.memset(expb_sb[:], -EXPC)
    # ones columns for v8 (column 64 of each 66-wide head block)
    v8_blocks = v8_sb[:].rearrange("p (jt h e) -> p jt h e", jt=NJT, e=VR)
    nc.vector.memset(v8_blocks[:, :, :, D : D + 1], 1.0)

    # --- stage emitters ---
    def qk_quarter(wsb, dst, bcol, t, c):
        """One (dst, chunk) quarter of q/k projection for f'-tile t."""
        p = ps.tile([128, 512], F32, tag="mm", bufs=2, name=f"qk{t}_{bcol}_{c}")
        for k in range(NK):
            nc.tensor.matmul(
                p[:],
                wsb[:, 512 * k + 128 * t : 512 * k + 128 * t + 128],
                x16_sb[:, N * k + 512 * c : N * k + 512 * c + 512],
                start=(k == 0),
                stop=(k == NK - 1),
            )
        nc.vector.tensor_scalar_add(
            dst[:, N * t + 512 * c : N * t + 512 * c + 512],
            p[:],
            bqk_sb[:, bcol : bcol + 1],
        )

    def qk_tile(t):
        # c0 quarters first: scores (h, jt<4) only need chunk 0 of qT AND kT
        for c in range(NCH):
            for wsb, dst, bcol in ((wq_sb, qT_sb, t), (wk_sb, kT_sb, 4 + t)):
                qk_quarter(wsb, dst, bcol, t, c)

    def v_tile(jt):
        """Project v for token tile jt: (128 tokens, 512 feats) -> v8 (fp8)."""
        p = ps.tile([128, 512], F32, tag="mm", bufs=2)
        for k in range(NK):
            nc.tensor.matmul(
                p[:],
                x16_sb[:, N * k + 128 * jt : N * k + 128 * jt + 128],
                wv_sb[:, 512 * k : 512 * k + 512],
                start=(k == 0),
                stop=(k == NK - 1),
            )
        nc.vector.tensor_copy(
            v8_blocks[:, jt, :, 0:D],
            p[:].rearrange("p (h e) -> p h e", e=D),
        )

    pt_tiles = {}
    pt_done = {}  # head -> number of j-tile exps emitted

    def scores_exp(h, jt):
        """scoresT (j, i) for head h, j-tile jt + exp -> pT fp8e4.

        Two heads (hh = h%2) run concurrently in PE row-quadrants via
        tile_position; ACT applies exp(s - EXPC) straight off PSUM."""
        pr, hh = divmod(h, 2)
        if h in pt_tiles:
            pT = pt_tiles[h]
        else:
            pT = pt_pool.tile([128, NJT * N], F8, tag="pt", name=f"pt{h}")
            pt_tiles[h] = pT
        pt_done[h] = jt + 1  # exps emitted so far for this head
        po = 64 * hh
        sp = ps.tile([128, N], F32, tag="score", bufs=3)
        for c in range(NCH):
            nc.tensor.matmul(
                sp[:, 512 * c : 512 * c + 512],
                kT_sb[po : po + 64, N * pr + 128 * jt : N * pr + 128 * jt + 128],
                qT_sb[po : po + 64, N * pr + 512 * c : N * pr + 512 * c + 512],
                start=True,
                stop=True,
                tile_position=(po, 0),
            )
        nc.scalar.activation(
            pT[:, N * jt : N * jt + N], sp[:], AF.Exp, bias=expb_sb[:]
        )

    pv_tiles = {}

    def attnv_unit(h, c, g):
        """One fp8 DoubleRow matmul (j-tiles 2g, 2g+1) of outT~ for (h, c);
        denominator chain + os8 scaling after the last unit of the chunk."""
        pr, hh = divmod(h, 2)
        pT = pt_tiles[h]
        key = (h, c)
        if key not in pv_tiles:
            pv_tiles[key] = ps.tile(
                [128, 512], F32, tag="mm", bufs=2, name=f"pv{h}_{c}"
            )
        p = pv_tiles[key]
        if USE_DR_ATTNV:
            lhs = v8_sb[:].rearrange(
                "p (jp two h e) -> p jp two h e", jp=NJT // 2, two=2, e=VR
            )[:, g, :, h, 0 : D + 1]
            rhs = pT[:].rearrange("p (jp two i) -> p jp two i", jp=NJT // 2, two=2)[
                :, g, :, 512 * c : 512 * c + 512
            ]
            nc.tensor.matmul(
                p[0:65, :],
                lhs,
                rhs,
                start=(g == 0),
                stop=(g == NJT // 2 - 1),
                perf_mode=PM.DoubleRow,
            )
        else:
            vb = v8_sb[:].rearrange("p (jt h e) -> p jt h e", jt=NJT, e=VR)
            for jt in (2 * g, 2 * g + 1):
                nc.tensor.matmul(
                    p[0:65, :],
                    vb[:, jt, h, 0 : D + 1],
                    pT[:, N * jt + 512 * c : N * jt + 512 * c + 512],
                    start=(jt == 0),
                    stop=(jt == NJT - 1),
                )
        if g == NJT // 2 - 1:
            del pv_tiles[key]
            r = 2 * h + c
            # denominator chain: approx-reciprocal straight off the PSUM
            # denom row (p64) -> partition 0, gpsimd broadcast to 64
            # partitions, one multiply off PSUM into os8 (fp8).
            rrow = rr_pool.tile([1, 512], F32, tag="rr", name=f"rr{r}")
            if USE_RECIP_FAST:
                nc.vector.reciprocal_approx_fast(rrow[0:1, :], p[64:65, :])
            else:
                nc.vector.reciprocal(rrow[0:1, :], p[64:65, :])
            rb = rb_pool.tile([64, 512], F32, tag="rb", name=f"rb{r}")
            nc.gpsimd.partition_broadcast(rb[:], rrow[0:1, :])
            sl = slice(N * pr + 512 * c, N * pr + 512 * c + 512)
            nc.vector.tensor_tensor(
                os_sb[64 * hh : 64 * hh + 64, sl],
                p[0:64, :],
                rb[0:64, :],
                ALU.mult,
            )

    def op_half(ct, c, up):
        """Out-proj k-tile pair `up` for output tile (ct, c): 2 fp8 DoubleRow
        matmuls into a 1-bank psum. Pair 0 (heads 0-3) runs mid-kernel and
        banks its partial + residual into yA; pair 1 adds the rest at the
        tail and drains to DRAM."""
        p = ps.tile([128, 512], F32, tag="mm", bufs=2, name=f"op{ct}_{c}_{up}")
        if USE_DR_OUTPROJ:
            wl3 = wl8_sb[:].rearrange("p (up two km) -> p up two km", up=2, two=2)
            os3 = os_sb[:].rearrange("p (up two i) -> p up two i", up=2, two=2)
            nc.tensor.matmul(
                p[:],
                wl3[:, up, :, 128 * ct : 128 * ct + 128],
                os3[:, up, :, 512 * c : 512 * c + 512],
                start=True,
                stop=True,
                perf_mode=PM.DoubleRow,
            )
        else:
            for k in (2 * up, 2 * up + 1):
                nc.tensor.matmul(
                    p[:],
                    wl8_sb[:, 512 * k + 128 * ct : 512 * k + 128 * ct + 128],
                    os_sb[:, N * k + 512 * c : N * k + 512 * c + 512],
                    start=(k == 2 * up),
                    stop=(k == 2 * up + 1),
                )
        sl = slice(N * ct + 512 * c, N * ct + 512 * c + 512)
        if up == 0:
            nc.vector.tensor_tensor(yA_sb[:, sl], p[:], xr_sb[:, sl], ALU.add)
        else:
            y = y_pool.tile([128, 512], F32, tag="y", name=f"y{ct}_{c}")
            nc.vector.tensor_tensor(y[:], p[:], yA_sb[:, sl], ALU.add)
            q = (nc.sync, nc.scalar, nc.gpsimd)[(2 * ct + c) % 3]
            q.dma_start(
                d["y"].ap()[128 * ct : 128 * ct + 128, 512 * c : 512 * c + 512], y[:]
            )

    # --- pools that emitters close over ---
    import contextlib

    stack = contextlib.ExitStack()
    pt_pool = stack.enter_context(tc.tile_pool(name="pt", bufs=3))
    rr_pool = stack.enter_context(tc.tile_pool(name="rr", bufs=3))
    rb_pool = stack.enter_context(tc.tile_pool(name="rb", bufs=3))
    y_pool = stack.enter_context(tc.tile_pool(name="y", bufs=4))

    # --- software-pipelined emission ---
    # unit stream drained behind the ACT exp stream: v tiles first (needed by
    # the first attnv), then attnv DoubleRow units, g-major per head so a
    # unit only waits for its own two pT j-tiles (exps jt <= 2g+1). Out-proj
    # pair-0 halves slot in right after head 3's attnv completes.
    units = [("v", jt) for jt in range(2, NJT)]
    for h in range(H):
        units += [("av", h, c, g) for g in range(4) for c in range(NCH)]
        if h == 3:
            units += [("op0", ct, c) for ct in range(4) for c in range(NCH)]
    upos = 0

    def drain(n):
        nonlocal upos
        done = 0
        while done < n and upos < len(units):
            u = units[upos]
            # attnv unit (h, c, g) needs exps for j-tiles 2g, 2g+1 of head h
            if u[0] == "av" and pt_done.get(u[1], 0) < 2 * u[3] + 2:
                break
            upos += 1
            done += 1
            if u[0] == "v":
                v_tile(u[1])
            elif u[0] == "op0":
                op_half(u[1], u[2], 0)
            else:
                attnv_unit(u[1], u[2], u[3])

    def drain_ready(n):
        """Drain up to n units that are dependency-ready (never blocks)."""
        drain(n)

    qk_tile(0)
    qk_tile(1)
    qk_quarters = [
        (wsb, dst, bcol, t, c)
        for t in (2, 3)
        for (wsb, dst, bcol) in ((wq_sb, qT_sb, t), (wk_sb, kT_sb, 4 + t))
        for c in range(NCH)
    ]
    for g in range(NJT):  # head 0 scores + qk tiles 2,3 (one quarter per step)
        scores_exp(0, g)
        wsb, dst, bcol, t, c = qk_quarters[g]
        qk_quarter(wsb, dst, bcol, t, c)
    for g in range(NJT):  # head 1 scores + first v tiles
        scores_exp(1, g)
        if g < 2:
            v_tile(g)
        drain_ready(1)
    for h in range(2, H):
        for g in range(NJT):
            scores_exp(h, g)
            drain_ready(2)
    drain(len(units))  # remainder (attnv of heads 6,7 + last denoms)
    for ct in range(4):
        for c in range(NCH):
            op_half(ct, c, 1)

    stack.close()


def _build(loop=1):
    nc = bacc.Bacc("TRN2", target_bir_lowering=False, debug=False, num_devices=BS)
    d = {}
    d["x16"] = nc.dram_tensor("x16", [CIN, N], F16, kind="ExternalInput")
    d["xr"] = nc.dram_tensor("xr", [CIN, N], F32, kind="ExternalInput")
    d["wq"] = nc.dram_tensor("wq", [CIN, 512], F16, kind="ExternalInput")
    d["wk"] = nc.dram_tensor("wk", [CIN, 512], F16, kind="ExternalInput")
    d["wv"] = nc.dram_tensor("wv", [CIN, 512], F16, kind="ExternalInput")
    d["wl8"] = nc.dram_tensor("wl8", [CIN, 512], F8, kind="ExternalInput")
    d["bqk"] = nc.dram_tensor("bqk", [128, 8], F32, kind="ExternalInput")
    d["y"] = nc.dram_tensor("y", [CIN, N], F32, kind="ExternalOutput")

    with tile.TileContext(nc) as tc:
        with (
            tc.tile_pool(name="sb", bufs=1) as sb,
            tc.tile_pool(name="ps", bufs=1, space="PSUM") as ps,
        ):
            for i in range(loop):
                if i:
                    with tc.tile_critical():
                        nc.all_engine_barrier()
                _emit(tc, d, sb, ps)
    nc.compile()
    return nc


_NC_CACHE = {}


def get_nc(loop=1):
    if loop not in _NC_CACHE:
        _NC_CACHE[loop] = _build(loop)
    return _NC_CACHE[loop]


def host_prep(x, W_fc, b_fc, W_last, b_last):
    """Full inputs -> list of 8 per-core input maps."""
    x = np.asarray(x, dtype=np.float64)
    W_fc = np.asarray(W_fc, dtype=np.float64)
    b_fc = np.asarray(b_fc, dtype=np.float64)
    W_last = np.asarray(W_last, dtype=np.float64)
    b_last = np.asarray(b_last, dtype=np.float64)

    hh = np.arange(H).repeat(D) * 3 * D  # 192h per f'=64h+d
    dd = np.tile(np.arange(D), H)
    pq, pk, pv = hh + dd, hh + D + dd, hh + 2 * D + dd

    wq = np.ascontiguousarray((W_fc[pq] * 0.125).T).astype(np.float16)
    wk = np.ascontiguousarray(W_fc[pk].T).astype(np.float16)
    wv = np.ascontiguousarray(W_fc[pv].T).astype(np.float16)
    wl8 = np.ascontiguousarray(W_last.T).astype(ml_dtypes.float8_e4m3)
    bq, bk, bv = b_fc[pq] * 0.125, b_fc[pk], b_fc[pv]
    bqk = np.ascontiguousarray(
        np.concatenate([bq.reshape(4, 128).T, bk.reshape(4, 128).T], axis=1)
    ).astype(np.float32)
    # b_v passes through softmax unchanged (weights sum to 1): fold W_last@bv
    # into the residual bias.
    b_eff = b_last + W_last @ bv

    xf = x.reshape(BS, CIN, N)
    maps = []
    for b in range(BS):
        maps.append(
            {
                "x16": xf[b].astype(np.float16),
                "xr": (xf[b] + b_eff[:, None]).astype(np.float32),
                "wq": wq,
                "wk": wk,
                "wv": wv,
                "wl8": wl8,
                "bqk": bqk,
            }
        )
    return maps


def kernel(x, W_fc, b_fc, W_last, b_last):
    nc = get_nc()
    maps = host_prep(x, W_fc, b_fc, W_last, b_last)
    res = run_bass_kernel_spmd(nc, maps, core_ids=list(range(BS)))
    y = np.stack([res.results[b]["y"] for b in range(BS)])
    return y.reshape(BS, CIN, 32, 32)
